# revision 42
# baseline (speedup 1.0000x reference)
"""Trainium2 Bass kernel for nn_BaseGR (2-layer hetero-SAGE GNN + predictor).

8-core strategy:
  - Users sharded 12500/core, items sharded 2500/core (padded blocks of
    2560); group rows replicated via partial sums + AllReduce.
  - Each segment-mean: dma_gather of neighbor feature rows (bf16, HBM) ->
    one-hot built on DVE (iota==dst_local)*weight -> TensorE scatter-matmul
    accumulating [H, dst_tile] in PSUM -> W-matmul.
  - User/group tables store BOTH layers' features per 512B row
    ([h0 | h1]), so one gather serves layer 1 and layer 2 (the gather cost
    is per-row latency-bound, so doubling the row size is ~free).
  - oi1 partials are ReduceScattered item-major (rank c receives exactly
    its item shard); og1/og2 partials share one bf16 AllReduce.
  - Final predictor computed transposed ([items, groups]) so pred_b is a
    per-partition bias; host returns a zero-cost .T view.
"""

import sys

sys.path.insert(0, "/opt/trn_rl_repo")

import numpy as np
import ml_dtypes

import concourse.bass as bass
import concourse.bacc as bacc
import concourse.mybir as mybir
import concourse.tile as tile
from concourse.bass_utils import run_bass_kernel_spmd
from concourse.alu_op_type import AluOpType

BF16 = ml_dtypes.bfloat16
F32 = np.float32

NG, NU, NI, H = 5000, 100000, 20000, 128
W = 8
USH = NU // W            # 12500 users per core
USH_P = 12544            # padded (98 tiles)
ISH = NI // W            # 2500 items per core
ISH_P = 2560             # padded (20 tiles)
NI_P = ISH_P * W         # 20480 padded item space
NG_P = 5120              # padded groups (40 tiles)
N_UT = USH_P // 128      # 98 user tiles
N_IT = NI_P // 128       # 160 item tiles (padded space)
N_GT = NG_P // 128       # 40 group tiles
N_IST = ISH_P // 128     # 20 local item tiles
SEG_UG = 16              # chunks per gather call (512B rows)
SEG_SM = 24              # chunks per gather call (256B rows)


def _pad_item(i):
    return (i // ISH) * ISH_P + (i % ISH)


class Dir:
    """One gather/scatter direction. Structure (tiles/segments/chunk counts)
    is shared by all cores; index/weight arrays are per-core."""

    def __init__(self, name, n_dst_tiles, force_all_tiles, seg_chunks):
        self.name = name
        self.n_dst_tiles = n_dst_tiles
        self.force_all_tiles = force_all_tiles
        self.seg_chunks = seg_chunks
        self.tiles = []      # [(tile_id, chunk_ofs, n_chunks)]
        self.segments = []   # [(chunk_start, n_chunks, [tile entries])]
        self.total_chunks = 0
        self.idx = None      # [W, 128, C*8] int16 (16-wrapped, replicated)
        self.dstl = None     # [W, 128, C] bf16
        self.wv = None       # [W, 128, C] bf16

    def build(self, per_core):
        ncore = len(per_core)
        buckets = [[None] * self.n_dst_tiles for _ in range(ncore)]
        for c, (gidx, dst, wgt) in enumerate(per_core):
            t = dst // 128
            order = np.argsort(t, kind="stable")
            t_s = t[order]
            bounds = np.searchsorted(t_s, np.arange(self.n_dst_tiles + 1))
            for ti in range(self.n_dst_tiles):
                sl = order[bounds[ti]:bounds[ti + 1]]
                if len(sl):
                    # ascending gather addresses within the tile: the SDMA
                    # round trips are latency-bound; locality helps row hits
                    buckets[c][ti] = sl[np.argsort(gidx[sl], kind="stable")]
        n_chunks = np.zeros(self.n_dst_tiles, np.int64)
        for ti in range(self.n_dst_tiles):
            mx = max(len(buckets[c][ti]) if buckets[c][ti] is not None else 0
                     for c in range(ncore))
            if mx == 0 and self.force_all_tiles:
                mx = 1
            n_chunks[ti] = (mx + 127) // 128 if mx else 0
        ofs = 0
        seg_start, seg_n, seg_tiles = 0, 0, []
        for ti in range(self.n_dst_tiles):
            nc_t = int(n_chunks[ti])
            if nc_t == 0:
                continue
            if seg_n and seg_n + nc_t > self.seg_chunks:
                self.segments.append((seg_start, seg_n, seg_tiles))
                seg_start, seg_n, seg_tiles = ofs, 0, []
            self.tiles.append((ti, ofs, nc_t))
            seg_tiles.append((ti, ofs, nc_t))
            ofs += nc_t
            seg_n += nc_t
        if seg_n:
            self.segments.append((seg_start, seg_n, seg_tiles))
        self.total_chunks = ofs

        C = self.total_chunks
        self.idx = np.zeros((ncore, 128, C * 8), np.int16)
        self.dstl = np.zeros((ncore, 128, C), BF16)
        self.wv = np.zeros((ncore, 128, C), BF16)
        for c, (gidx, dst, wgt) in enumerate(per_core):
            i1 = np.zeros(C * 128, np.int16)
            dl = np.zeros(C * 128, F32)
            wv = np.zeros(C * 128, F32)
            for (ti, ofs_t, nct) in self.tiles:
                sl = buckets[c][ti]
                if sl is None:
                    continue
                n = len(sl)
                base = ofs_t * 128
                i1[base:base + n] = gidx[sl]
                dl[base:base + n] = (dst[sl] - ti * 128).astype(F32)
                wv[base:base + n] = wgt[sl]
            for (cs, cn, _st) in self.segments:
                blk = i1[cs * 128:(cs + cn) * 128].reshape(16, cn * 8, order="F")
                self.idx[c][:, cs * 8:(cs + cn) * 8] = np.tile(blk, (8, 1))
            self.dstl[c] = dl.reshape(C, 128).T.astype(BF16)
            self.wv[c] = wv.reshape(C, 128).T.astype(BF16)


def _prep(inputs):
    x_user = np.asarray(inputs["x_user"])
    x_item = np.asarray(inputs["x_item"])
    hu0 = np.asarray(inputs["emb_user"], F32)[x_user]
    hi0 = np.asarray(inputs["emb_item"], F32)[x_item]
    W1l = np.asarray(inputs["W1l"], F32)
    W1r = np.asarray(inputs["W1r"], F32)
    b1 = np.asarray(inputs["b1"], F32)
    W2l = np.asarray(inputs["W2l"], F32)
    W2r = np.asarray(inputs["W2r"], F32)
    b2 = np.asarray(inputs["b2"], F32)
    predW = np.asarray(inputs["pred_W"], F32)
    predb = np.asarray(inputs["pred_b"], F32)
    ug_src = np.asarray(inputs["ug_src"], np.int64)
    ug_dst = np.asarray(inputs["ug_dst"], np.int64)
    ui_src = np.asarray(inputs["ui_src"], np.int64)
    ui_dst = np.asarray(inputs["ui_dst"], np.int64)
    gi_src = np.asarray(inputs["gi_src"], np.int64)
    gi_dst = np.asarray(inputs["gi_dst"], np.int64)

    w_ug_g = (1.0 / np.maximum(np.bincount(ug_dst, minlength=NG), 1)).astype(F32)
    w_gi_g = (1.0 / np.maximum(np.bincount(gi_src, minlength=NG), 1)).astype(F32)
    w_ui_i = (1.0 / np.maximum(np.bincount(ui_dst, minlength=NI), 1)).astype(F32)
    w_ui_u = (1.0 / np.maximum(np.bincount(ui_src, minlength=NU), 1)).astype(F32)

    # user table [USH_P, 256]: cols 0:128 = hu0 shard; 128:256 = hu1 (device)
    ugt = np.zeros((W, USH_P, 2 * H), BF16)
    # item shard table [ISH_P, 256]: cols 0:128 = hi0 shard; 128:256 = hi1
    git = np.zeros((W, ISH_P, 2 * H), BF16)
    # full item table (layer1 features only) for i2u gathers
    ite = np.zeros((NI_P, H), BF16)
    for c in range(W):
        ugt[c, :USH, :H] = hu0[c * USH:(c + 1) * USH].astype(BF16)
        git[c, :ISH, :H] = hi0[c * ISH:(c + 1) * ISH].astype(BF16)
        ite[c * ISH_P:c * ISH_P + ISH] = hi0[c * ISH:(c + 1) * ISH].astype(BF16)

    d_ug = Dir("ug", N_GT, False, SEG_UG)
    per = []
    for c in range(W):
        m = (ug_src >= c * USH) & (ug_src < (c + 1) * USH)
        per.append(((ug_src[m] - c * USH).astype(np.int16),
                    ug_dst[m], w_ug_g[ug_dst[m]]))
    d_ug.build(per)

    # gi is dense enough (25K edges onto 2560x5120 per core) that a
    # host-built adjacency block beats per-edge gathers 4x.
    agi = np.zeros((W, ISH_P, NG_P), BF16)
    for c in range(W):
        m = (gi_dst >= c * ISH) & (gi_dst < (c + 1) * ISH)
        il = (gi_dst[m] - c * ISH).astype(np.int64)
        g = gi_src[m]
        acc = np.zeros((ISH_P, NG_P), F32)
        np.add.at(acc, (il, g), w_gi_g[g])
        agi[c] = acc.astype(BF16)

    d_uii = Dir("uii", N_IT, True, SEG_SM)   # u2i: dst = items (padded)
    d_iu = Dir("iu", N_UT, True, SEG_SM)     # i2u: dst = local users
    per_uii, per_iu = [], []
    for c in range(W):
        m = (ui_src >= c * USH) & (ui_src < (c + 1) * USH)
        us, ud = ui_src[m], ui_dst[m]
        per_uii.append(((us - c * USH).astype(np.int16),
                        _pad_item(ud), w_ui_i[ud]))
        per_iu.append((_pad_item(ud).astype(np.int16),
                       (us - c * USH), w_ui_u[us]))
    d_uii.build(per_uii)
    d_iu.build(per_iu)

    wts = np.stack([
        W1l[0], W1l[5],                 # og1: u2g, i2g
        W1l[2], W1r[2] + W1r[4],        # oi1: u2i agg, dense
        W1l[3], W1r[1] + W1r[3],        # ou1: i2u agg, dense
        W2l[0], W2l[5], W2r[0] + W2r[5]  # og2
    ]).astype(BF16)
    biases = np.stack([b1[0] + b1[5], b1[1] + b1[3],
                       b2[0] + b2[5], np.zeros(H, F32)], axis=1).astype(F32)
    btile_i1 = np.broadcast_to((b1[2] + b1[4]).astype(BF16), (128, H)).copy()
    ident = np.eye(128, dtype=BF16)
    iota = np.broadcast_to(np.arange(128, dtype=np.float32),
                           (128, 128)).astype(BF16)

    predW_sh = np.zeros((W, H, ISH_P), BF16)
    predb_sh = np.zeros((W, N_IST, 128), F32)
    for c in range(W):
        predW_sh[c][:, :ISH] = predW[:, c * ISH:(c + 1) * ISH].astype(BF16)
        pb = np.zeros(ISH_P, F32)
        pb[:ISH] = predb[c * ISH:(c + 1) * ISH]
        predb_sh[c] = pb.reshape(N_IST, 128)

    in_maps = []
    for c in range(W):
        in_maps.append({
            "ugt": ugt[c], "git": git[c], "ite": ite,
            "wts": wts, "biases": biases, "btile_i1": btile_i1,
            "ident": ident, "iota": iota,
            "predw": predW_sh[c], "predb": predb_sh[c],
            "ug_idx": d_ug.idx[c], "ug_dstl": d_ug.dstl[c], "ug_wv": d_ug.wv[c],
            "agi": agi[c],
            "uii_idx": d_uii.idx[c], "uii_dstl": d_uii.dstl[c],
            "uii_wv": d_uii.wv[c],
            "iu_idx": d_iu.idx[c], "iu_dstl": d_iu.dstl[c], "iu_wv": d_iu.wv[c],
        })
    return in_maps, {"ug": d_ug, "uii": d_uii, "iu": d_iu}


def _build(struct):
    d_ug, d_uii, d_iu = struct["ug"], struct["uii"], struct["iu"]
    nc = bacc.Bacc("TRN2", target_bir_lowering=False, num_swdge_queues=4)
    bf = mybir.dt.bfloat16
    f32 = mybir.dt.float32
    i16 = mybir.dt.int16

    P = {}

    def param(name, shape, dt):
        P[name] = nc.declare_dram_parameter(name, list(shape), dt,
                                            isOutput=False)
        return P[name]

    ugt = param("ugt", [USH_P, 2 * H], bf)
    git = param("git", [ISH_P, 2 * H], bf)
    ite = param("ite", [NI_P, H], bf)
    wts = param("wts", [9, 128, 128], bf)
    biases = param("biases", [128, 4], f32)
    btile_i1 = param("btile_i1", [128, H], bf)
    ident_d = param("ident", [128, 128], bf)
    iota_d = param("iota", [128, 128], bf)
    predw = param("predw", [H, ISH_P], bf)
    predb = param("predb", [N_IST, 128], f32)
    agi_d = param("agi", [ISH_P, NG_P], bf)
    for nm, d in (("ug", d_ug), ("uii", d_uii), ("iu", d_iu)):
        C = d.total_chunks
        param(f"{nm}_idx", [128, C * 8], i16)
        param(f"{nm}_dstl", [128, C], bf)
        param(f"{nm}_wv", [128, C], bf)
    outp = nc.declare_dram_parameter("out", [ISH_P, NG], bf, isOutput=True)

    with tile.TileContext(nc) as tc:
        with (
            tc.tile_pool(name="cst", bufs=1) as cst,
            tc.tile_pool(name="gp", bufs=2) as gp,
            tc.tile_pool(name="sp", bufs=3) as sp,
            tc.tile_pool(name="st", bufs=2) as stp,
            tc.tile_pool(name="big", bufs=2) as bigp,
            tc.tile_pool(name="psum", bufs=1, space="PSUM") as psum,
            tc.tile_pool(name="dram", bufs=1, space="DRAM") as dram,
        ):
            wt_sb = []
            for k in range(9):
                t = cst.tile([128, 128], bf, tag=f"w{k}")
                nc.sync.dma_start(t[:], wts[k])
                wt_sb.append(t)
            (W_og_u, W_og_i, W_oi_a, W_oi_d, W_ou_a, W_ou_d,
             W_og2_u, W_og2_i, W_og2_d) = wt_sb
            bias_sb = cst.tile([128, 4], f32, tag="bias")
            nc.sync.dma_start(bias_sb[:], biases[:])
            bti_sb = cst.tile([128, H], bf, tag="bti")
            nc.sync.dma_start(bti_sb[:], btile_i1[:])
            ident_sb = cst.tile([128, 128], bf, tag="ident")
            nc.sync.dma_start(ident_sb[:], ident_d[:])
            predb_sb = cst.tile([128, N_IST], f32, tag="predb")
            nc.sync.dma_start(predb_sb[:], predb[:].rearrange("a b -> b a"))
            iota_sb = cst.tile([128, 128], bf, tag="iota")
            nc.sync.dma_start(iota_sb[:], iota_d[:])

            darr = {}
            for nm, d in (("ug", d_ug), ("uii", d_uii), ("iu", d_iu)):
                C = d.total_chunks
                ti_ = cst.tile([128, C * 8], i16, tag=f"{nm}_idx")
                nc.sync.dma_start(ti_[:], P[f"{nm}_idx"][:])
                td = cst.tile([128, C], bf, tag=f"{nm}_dstl")
                nc.sync.dma_start(td[:], P[f"{nm}_dstl"][:])
                tw = cst.tile([128, C], bf, tag=f"{nm}_wv")
                nc.sync.dma_start(tw[:], P[f"{nm}_wv"][:])
                darr[nm] = (ti_, td, tw)

            ogT = bigp.tile([128, 2 * NG_P], bf, tag="big", name="ogT",
                            bufs=1)
            nc.vector.memset(ogT[:], 0.0)
            og1T = ogT[:, 0:NG_P]
            og2T = ogT[:, NG_P:2 * NG_P]
            hg1T = cst.tile([128, NG_P], bf, tag="hg1T")
            repT = cst.tile([128, NG_P], bf, tag="repT")

            hiT_full = cst.tile([128, ISH_P], bf, tag="hiTf")
            nc.sync.dma_start(hiT_full[:], git[:, 0:H], transpose=True)

            aroi_in = dram.tile([NI_P, H], bf)
            rs_oi = dram.tile([ISH_P, H], bf)
            ar1_in = dram.tile([128, NG_P], bf)
            ar1_out = dram.tile([128, NG_P], bf)
            ar2_in = dram.tile([128, NG_P], bf)
            ar2_out = dram.tile([128, NG_P], bf)

            qctr = [0]

            def segsum(d, table_ap, elem_size, elem_step, width, out_cb,
                       between=None):
                idx_sb, dstl_sb, wv_sb = darr[d.name]
                for si, (cs, cn, seg_tiles) in enumerate(d.segments):
                    if between is not None:
                        between(si)
                    gt = gp.tile([128, d.seg_chunks, width], bf,
                                 tag=f"gath{width}",
                                 bufs=(4 if width == H else 3))
                    n_idx = cn * 128
                    nc.gpsimd.dma_gather(
                        gt[:, :cn, :], table_ap,
                        idx_sb[:, cs * 8:(cs + cn) * 8],
                        n_idx, n_idx, elem_size, elem_step=elem_step,
                        single_packet=False, queue_num=qctr[0] % 4)
                    qctr[0] += 1
                    oh = gp.tile([128, d.seg_chunks, 128], bf, tag="oh",
                                 bufs=2)
                    iota_b = (iota_sb[:].rearrange("p (o k) -> p o k", o=1)
                              .to_broadcast((128, cn, 128)))
                    dstl_b = (dstl_sb[:, cs:cs + cn]
                              .rearrange("p (c o) -> p c o", o=1)
                              .to_broadcast((128, cn, 128)))
                    wv_b = (wv_sb[:, cs:cs + cn]
                            .rearrange("p (c o) -> p c o", o=1)
                            .to_broadcast((128, cn, 128)))
                    nc.vector.tensor_tensor(oh[:, :cn, :], iota_b, dstl_b,
                                            AluOpType.is_equal)
                    nc.vector.tensor_tensor(oh[:, :cn, :], oh[:, :cn, :],
                                            wv_b, AluOpType.mult)
                    for (ti, ofs_t, nct) in seg_tiles:
                        out_cb(ti, gt, oh, ofs_t - cs, nct)

            # ---------- Phase 1: i2u -> hu1 (local users) ----------
            scope_p1 = tc.spectator_scope("p1_i2u")
            scope_p1.__enter__()
            hu_stage = [None]

            huTg_cache = [None]

            def get_huT(ti):
                g8 = ti // 8
                if huTg_cache[0] is None or huTg_cache[0][0] != g8:
                    n_t = min(8, N_UT - g8 * 8)
                    tl = sp.tile([128, 1024], bf, tag="huTg", name="huTg", bufs=2)
                    nc.sync.dma_start(
                        tl[:, :n_t * 128],
                        ugt[g8 * 1024:g8 * 1024 + n_t * 128, 0:H],
                        transpose=True)
                    huTg_cache[0] = (g8, tl)
                return huTg_cache[0][1][:, (ti % 8) * 128:(ti % 8 + 1) * 128]

            def cb_ou(ti, gt, oh, lc0, nct):
                ps = psum.tile([128, 128], f32, tag="agg", bufs=2)
                for j in range(nct):
                    nc.tensor.matmul(ps[:], gt[:, lc0 + j, :], oh[:, lc0 + j, :],
                                     start=(j == 0), stop=(j == nct - 1))
                aggT = sp.tile([128, 128], bf, tag="aggT", bufs=4)
                nc.scalar.activation(aggT[:], ps[:],
                                     mybir.ActivationFunctionType.Copy)
                pw = psum.tile([128, 128], f32, tag="w", bufs=2)
                nc.tensor.matmul(pw[:], W_ou_a[:], aggT[:], start=True,
                                 stop=False)
                nc.tensor.matmul(pw[:], W_ou_d[:], get_huT(ti), start=False,
                                 stop=True)
                ouT = sp.tile([128, 128], bf, tag="ouT", bufs=4)
                nc.scalar.activation(ouT[:], pw[:],
                                     mybir.ActivationFunctionType.Relu,
                                     bias=bias_sb[:, 1:2])
                ptr = psum.tile([128, 128], bf, tag="w", bufs=2)
                nc.tensor.transpose(ptr[:], ouT[:], ident_sb[:])
                g, s = ti // 16, ti % 16
                if hu_stage[0] is None:
                    hu_stage[0] = stp.tile([128, 16, 128], bf, tag="hust", name="hust")
                nc.vector.tensor_copy(hu_stage[0][:, s, :], ptr[:])
                if s == 15 or ti == N_UT - 1:
                    n_g = s + 1
                    nc.sync.dma_start(
                        ugt[g * 2048:g * 2048 + n_g * 128, H:2 * H]
                        .rearrange("(k p) h -> p k h", p=128),
                        hu_stage[0][:, :n_g, :])
                    hu_stage[0] = None

            segsum(d_iu, ite[:], H, H, H, cb_ou)
            scope_p1.__exit__(None, None, None)

            # ---------- Phase 2: u2i -> oi1 partial (item-major) ----------
            scope_p2 = tc.spectator_scope("p2_u2i")
            scope_p2.__enter__()
            oi_stage = [None]

            def cb_oi(ti, gt, oh, lc0, nct):
                ps = psum.tile([128, 128], f32, tag="agg", bufs=2)
                for j in range(nct):
                    nc.tensor.matmul(ps[:], gt[:, lc0 + j, :], oh[:, lc0 + j, :],
                                     start=(j == 0), stop=(j == nct - 1))
                aggT = sp.tile([128, 128], bf, tag="aggT", bufs=4)
                nc.scalar.activation(aggT[:], ps[:],
                                     mybir.ActivationFunctionType.Copy)
                pw = psum.tile([128, 128], f32, tag="w", bufs=2)
                nc.tensor.matmul(pw[:], aggT[:], W_oi_a[:], start=True,
                                 stop=True)
                g, s = ti // 16, ti % 16
                if oi_stage[0] is None:
                    oi_stage[0] = stp.tile([128, 16, 128], bf, tag="oist", name="oist")
                nc.vector.tensor_copy(oi_stage[0][:, s, :], pw[:])
                if s == 15 or ti == N_IT - 1:
                    n_g = s + 1
                    nc.sync.dma_start(
                        aroi_in[g * 2048:g * 2048 + n_g * 128, :]
                        .rearrange("(k p) h -> p k h", p=128),
                        oi_stage[0][:, :n_g, :])
                    oi_stage[0] = None

            segsum(d_uii, ugt[:, 0:H], H, 2 * H, H, cb_oi)
            scope_p2.__exit__(None, None, None)

            with tc.spectator_scope("p2b_rs"):
                nc.gpsimd.collective_compute(
                    "ReduceScatter", AluOpType.add,
                    replica_groups=[list(range(W))],
                    ins=[aroi_in.opt()], outs=[rs_oi.opt()])

            # ---------- Phases 4a: u2g both layers (needs hu1 only) ------
            def make_cb_g(W_l1, W_l2):
                def cb(ti, gt, oh, lc0, nct):
                    ps0 = psum.tile([128, 128], f32, tag="agg", bufs=2)
                    ps1 = psum.tile([128, 128], f32, tag="agg1", bufs=2)
                    for j in range(nct):
                        nc.tensor.matmul(ps0[:], gt[:, lc0 + j, 0:H],
                                         oh[:, lc0 + j, :],
                                         start=(j == 0), stop=(j == nct - 1))
                        nc.tensor.matmul(ps1[:], gt[:, lc0 + j, H:2 * H],
                                         oh[:, lc0 + j, :],
                                         start=(j == 0), stop=(j == nct - 1))
                    a0 = sp.tile([128, 128], bf, tag="aggT", bufs=4)
                    nc.scalar.activation(a0[:], ps0[:],
                                         mybir.ActivationFunctionType.Copy)
                    a1 = sp.tile([128, 128], bf, tag="aggT2", bufs=4)
                    nc.scalar.activation(a1[:], ps1[:],
                                         mybir.ActivationFunctionType.Copy)
                    pw = psum.tile([128, 128], f32, tag="w", bufs=2)
                    nc.tensor.matmul(pw[:], W_l1[:], a0[:], start=True,
                                     stop=True)
                    sl = slice(ti * 128, (ti + 1) * 128)
                    nc.vector.tensor_tensor(og1T[:, sl], og1T[:, sl], pw[:],
                                            AluOpType.add)
                    pw2 = psum.tile([128, 128], f32, tag="w", bufs=2)
                    nc.tensor.matmul(pw2[:], W_l2[:], a1[:], start=True,
                                     stop=True)
                    nc.vector.tensor_tensor(og2T[:, sl], og2T[:, sl], pw2[:],
                                            AluOpType.add)
                return cb

            # ---------- Phase 3: hi1 = relu(rs + dense + b) ----------
            # emitted lazily inside the u2g interleave (after RS has had a
            # few u2g segments' time to complete) to avoid stalling PE.
            def p3_hi1():
                for t in range(N_IST):
                    rs_sb = sp.tile([128, 128], bf, tag="rs")
                    nc.sync.dma_start(rs_sb[:],
                                      rs_oi[t * 128:(t + 1) * 128, :])
                    pd = psum.tile([128, 128], f32, tag="w", bufs=2)
                    nc.tensor.matmul(pd[:],
                                     hiT_full[:, t * 128:(t + 1) * 128],
                                     W_oi_d[:], start=True, stop=True)
                    t1 = sp.tile([128, 128], bf, tag="t1")
                    nc.vector.tensor_tensor(t1[:], rs_sb[:], pd[:],
                                            AluOpType.add)
                    t2 = sp.tile([128, 128], bf, tag="t2")
                    nc.vector.tensor_tensor(t2[:], t1[:], bti_sb[:],
                                            AluOpType.add)
                    hi1_t = sp.tile([128, 128], bf, tag="hi1")
                    nc.scalar.activation(hi1_t[:], t2[:],
                                         mybir.ActivationFunctionType.Relu)
                    nc.sync.dma_start(git[t * 128:(t + 1) * 128, H:2 * H],
                                      hi1_t[:])

            # ---------- Phase 5: i2g both layers via dense adjacency ----
            # mean_T[H, groups] = sum_t git_tile[K=item,H].T @ A[item, groups];
            # then (mean @ W) folds in afterwards per 1024-col block.
            # Emitted per-jg, interleaved into the u2g segment loop so the
            # PE work fills the gather-drain window; loads go on the
            # Activation engine's HWDGE queue to keep Sync free for oh.
            def p5_block(jg):
                p0 = psum.tile([128, 1024], f32, tag="agg", bufs=2)
                p1 = psum.tile([128, 1024], f32, tag="agg", bufs=2)
                for t in range(N_IST):
                    gsb = sp.tile([128, 2 * H], bf, tag="gisb", bufs=2)
                    nc.scalar.dma_start(gsb[:], git[t * 128:(t + 1) * 128, :])
                    asb = sp.tile([128, 1024], bf, tag="agisb", bufs=2)
                    nc.scalar.dma_start(
                        asb[:],
                        agi_d[t * 128:(t + 1) * 128,
                              jg * 1024:(jg + 1) * 1024])
                    for q in range(2):
                        nc.tensor.matmul(
                            p0[:, q * 512:(q + 1) * 512], gsb[:, 0:H],
                            asb[:, q * 512:(q + 1) * 512],
                            start=(t == 0), stop=(t == N_IST - 1))
                        nc.tensor.matmul(
                            p1[:, q * 512:(q + 1) * 512], gsb[:, H:2 * H],
                            asb[:, q * 512:(q + 1) * 512],
                            start=(t == 0), stop=(t == N_IST - 1))
                # aggregate-T is now in psum; fold W via aggT copy + W-MM
                for k in range(8):
                    sl = slice(jg * 1024 + k * 128, jg * 1024 + (k + 1) * 128)
                    a0 = sp.tile([128, 128], bf, tag="aggT", bufs=4)
                    nc.scalar.activation(a0[:], p0[:, k * 128:(k + 1) * 128],
                                         mybir.ActivationFunctionType.Copy)
                    a1 = sp.tile([128, 128], bf, tag="aggT2", bufs=4)
                    nc.scalar.activation(a1[:], p1[:, k * 128:(k + 1) * 128],
                                         mybir.ActivationFunctionType.Copy)
                    pw = psum.tile([128, 128], f32, tag="w", bufs=2)
                    nc.tensor.matmul(pw[:], W_og_i[:], a0[:], start=True,
                                     stop=True)
                    nc.vector.tensor_tensor(og1T[:, sl], og1T[:, sl], pw[:],
                                            AluOpType.add)
                    pw2 = psum.tile([128, 128], f32, tag="w", bufs=2)
                    nc.tensor.matmul(pw2[:], W_og2_i[:], a1[:], start=True,
                                     stop=True)
                    nc.vector.tensor_tensor(og2T[:, sl], og2T[:, sl], pw2[:],
                                            AluOpType.add)

            n_ug_segs = len(d_ug.segments)
            p3_done = [False]
            p5_done = [0]

            def p5_interleave(si):
                if si >= 3 and not p3_done[0]:
                    p3_hi1()
                    p3_done[0] = True
                if p3_done[0]:
                    want = min(5, max(0, (si - 4) * 5)
                               // max(n_ug_segs - 6, 1))
                    while p5_done[0] < want:
                        p5_block(p5_done[0])
                        p5_done[0] += 1

            # ------- Phase 4a+5: u2g gathers with p3+i2g interleaved ------
            scope_p45 = tc.spectator_scope("p45_u2g_i2g")
            scope_p45.__enter__()
            segsum(d_ug, ugt[:], 2 * H, 2 * H, 2 * H,
                   make_cb_g(W_og_u, W_og2_u), between=p5_interleave)
            if not p3_done[0]:
                p3_hi1()
                p3_done[0] = True
            while p5_done[0] < 5:
                p5_block(p5_done[0])
                p5_done[0] += 1
            scope_p45.__exit__(None, None, None)

            # ---------- Phase 6: split AllReduce og1 then og2 ----------
            # og1's AR completes first; the hg1 relu + dense fold overlap
            # og2's AR transfer.
            scope_p6 = tc.spectator_scope("p6_ar")
            scope_p6.__enter__()
            nc.sync.dma_start(ar1_in[:], og1T)
            nc.gpsimd.collective_compute(
                "AllReduce", AluOpType.add,
                replica_groups=[list(range(W))],
                ins=[ar1_in.opt()], outs=[ar1_out.opt()])
            nc.sync.dma_start(ar2_in[:], og2T)
            nc.gpsimd.collective_compute(
                "AllReduce", AluOpType.add,
                replica_groups=[list(range(W))],
                ins=[ar2_in.opt()], outs=[ar2_out.opt()])
            ar1_sb = bigp.tile([128, NG_P], bf, tag="arh", name="ar1_sb",
                               bufs=2)
            nc.sync.dma_start(ar1_sb[:], ar1_out[:])
            nc.scalar.activation(hg1T[:], ar1_sb[:],
                                 mybir.ActivationFunctionType.Relu,
                                 bias=bias_sb[:, 0:1])
            ar2_sb = bigp.tile([128, NG_P], bf, tag="arh", name="ar2_sb",
                               bufs=2)
            nc.sync.dma_start(ar2_sb[:], ar2_out[:])
            for j in range(NG_P // 512):
                pf = psum.tile([128, 512], f32, tag="agg", bufs=2)
                nc.tensor.matmul(pf[:], W_og2_d[:],
                                 hg1T[:, j * 512:(j + 1) * 512],
                                 start=True, stop=True)
                tt = sp.tile([128, 512], bf, tag="o2t")
                nc.vector.tensor_tensor(
                    tt[:], ar2_sb[:, j * 512:(j + 1) * 512],
                    pf[:], AluOpType.add)
                nc.scalar.activation(repT[:, j * 512:(j + 1) * 512], tt[:],
                                     mybir.ActivationFunctionType.Relu,
                                     bias=bias_sb[:, 2:3])
            scope_p6.__exit__(None, None, None)

            # ---------- Phase 7: out[item, group] = predW.T @ repT + b ----
            scope_p7 = tc.spectator_scope("p7_pred")
            scope_p7.__enter__()
            for t in range(N_IST):
                pw_t = sp.tile([H, 128], bf, tag="pwt")
                nc.sync.dma_start(pw_t[:], predw[:, t * 128:(t + 1) * 128])
                for j in range((NG + 1023) // 1024):
                    wj = min(1024, NG - j * 1024)
                    pf = psum.tile([128, 1024], f32, tag="agg", bufs=2)
                    for q in range((wj + 511) // 512):
                        wq = min(512, wj - q * 512)
                        col = j * 1024 + q * 512
                        nc.tensor.matmul(
                            pf[:, q * 512:q * 512 + wq],
                            pw_t[:],
                            repT[:, col:col + wq], start=True, stop=True)
                    stg = stp.tile([128, 1024], bf, tag="fstage", bufs=3)
                    nc.vector.tensor_scalar(
                        stg[:, :wj], pf[:, :wj],
                        predb_sb[:, t:t + 1], None, AluOpType.add)
                    nc.sync.dma_start(
                        outp[t * 128:(t + 1) * 128, j * 1024:j * 1024 + wj],
                        stg[:, :wj])
            scope_p7.__exit__(None, None, None)
    nc.compile()
    return nc


def kernel(**inputs):
    in_maps, struct = _prep(inputs)
    nc = _build(struct)
    res = run_bass_kernel_spmd(nc, in_maps, list(range(W)))
    parts = [res.results[c]["out"][:ISH] for c in range(W)]
    full = np.concatenate(parts, axis=0).astype(np.float32)  # [NI, NG]
    return full.T  # [NG, NI] zero-copy view



# revision 47
# speedup vs baseline: 1.0711x; 1.0711x over previous
"""Trainium2 Bass kernel for nn_BaseGR (2-layer hetero-SAGE GNN + predictor).

8-core strategy:
  - Users sharded 12500/core, items sharded 2500/core (padded blocks of
    2560); group rows replicated via partial sums + AllReduce.
  - Each segment-mean: dma_gather of neighbor feature rows (bf16, HBM) ->
    one-hot built on DVE (iota==dst_local)*weight -> TensorE scatter-matmul
    accumulating [H, dst_tile] in PSUM -> W-matmul.
  - User/group tables store BOTH layers' features per 512B row
    ([h0 | h1]), so one gather serves layer 1 and layer 2 (the gather cost
    is per-row latency-bound, so doubling the row size is ~free).
  - oi1 partials are ReduceScattered item-major (rank c receives exactly
    its item shard); og1/og2 partials share one bf16 AllReduce.
  - Final predictor computed transposed ([items, groups]) so pred_b is a
    per-partition bias; host returns a zero-cost .T view.
"""

import sys

sys.path.insert(0, "/opt/trn_rl_repo")

import numpy as np
import ml_dtypes

import concourse.bass as bass
import concourse.bacc as bacc
import concourse.mybir as mybir
import concourse.tile as tile
from concourse.bass_utils import run_bass_kernel_spmd
from concourse.alu_op_type import AluOpType

BF16 = ml_dtypes.bfloat16
F32 = np.float32

NG, NU, NI, H = 5000, 100000, 20000, 128
W = 8
USH = NU // W            # 12500 users per core
USH_P = 12544            # padded (98 tiles)
ISH = NI // W            # 2500 items per core
ISH_P = 2560             # padded (20 tiles)
NI_P = ISH_P * W         # 20480 padded item space
NG_P = 5120              # padded groups (40 tiles)
N_UT = USH_P // 128      # 98 user tiles
N_IT = NI_P // 128       # 160 item tiles (padded space)
N_GT = NG_P // 128       # 40 group tiles
N_IST = ISH_P // 128     # 20 local item tiles
SEG_UG = 16              # chunks per gather call (512B rows)
SEG_SM = 24              # chunks per gather call (256B rows)


def _pad_item(i):
    return (i // ISH) * ISH_P + (i % ISH)


class Dir:
    """One gather/scatter direction. Structure (tiles/segments/chunk counts)
    is shared by all cores; index/weight arrays are per-core."""

    def __init__(self, name, n_dst_tiles, force_all_tiles, seg_chunks):
        self.name = name
        self.n_dst_tiles = n_dst_tiles
        self.force_all_tiles = force_all_tiles
        self.seg_chunks = seg_chunks
        self.tiles = []      # [(tile_id, chunk_ofs, n_chunks)]
        self.segments = []   # [(chunk_start, n_chunks, [tile entries])]
        self.total_chunks = 0
        self.idx = None      # [W, 128, C*8] int16 (16-wrapped, replicated)
        self.dstl = None     # [W, 128, C] bf16
        self.wv = None       # [W, 128, C] bf16

    def build(self, per_core):
        ncore = len(per_core)
        buckets = [[None] * self.n_dst_tiles for _ in range(ncore)]
        for c, (gidx, dst, wgt) in enumerate(per_core):
            t = dst // 128
            order = np.argsort(t, kind="stable")
            t_s = t[order]
            bounds = np.searchsorted(t_s, np.arange(self.n_dst_tiles + 1))
            for ti in range(self.n_dst_tiles):
                sl = order[bounds[ti]:bounds[ti + 1]]
                if len(sl):
                    # ascending gather addresses within the tile: the SDMA
                    # round trips are latency-bound; locality helps row hits
                    buckets[c][ti] = sl[np.argsort(gidx[sl], kind="stable")]
        n_chunks = np.zeros(self.n_dst_tiles, np.int64)
        for ti in range(self.n_dst_tiles):
            mx = max(len(buckets[c][ti]) if buckets[c][ti] is not None else 0
                     for c in range(ncore))
            if mx == 0 and self.force_all_tiles:
                mx = 1
            n_chunks[ti] = (mx + 127) // 128 if mx else 0
        ofs = 0
        seg_start, seg_n, seg_tiles = 0, 0, []
        for ti in range(self.n_dst_tiles):
            nc_t = int(n_chunks[ti])
            if nc_t == 0:
                continue
            if seg_n and seg_n + nc_t > self.seg_chunks:
                self.segments.append((seg_start, seg_n, seg_tiles))
                seg_start, seg_n, seg_tiles = ofs, 0, []
            self.tiles.append((ti, ofs, nc_t))
            seg_tiles.append((ti, ofs, nc_t))
            ofs += nc_t
            seg_n += nc_t
        if seg_n:
            self.segments.append((seg_start, seg_n, seg_tiles))
        self.total_chunks = ofs

        C = self.total_chunks
        self.idx = np.zeros((ncore, 128, C * 8), np.int16)
        self.dstl = np.zeros((ncore, 128, C), BF16)
        self.wv = np.zeros((ncore, 128, C), BF16)
        for c, (gidx, dst, wgt) in enumerate(per_core):
            i1 = np.zeros(C * 128, np.int16)
            dl = np.zeros(C * 128, F32)
            wv = np.zeros(C * 128, F32)
            for (ti, ofs_t, nct) in self.tiles:
                sl = buckets[c][ti]
                if sl is None:
                    continue
                n = len(sl)
                base = ofs_t * 128
                i1[base:base + n] = gidx[sl]
                dl[base:base + n] = (dst[sl] - ti * 128).astype(F32)
                wv[base:base + n] = wgt[sl]
            for (cs, cn, _st) in self.segments:
                blk = i1[cs * 128:(cs + cn) * 128].reshape(16, cn * 8, order="F")
                self.idx[c][:, cs * 8:(cs + cn) * 8] = np.tile(blk, (8, 1))
            self.dstl[c] = dl.reshape(C, 128).T.astype(BF16)
            self.wv[c] = wv.reshape(C, 128).T.astype(BF16)


def _prep(inputs):
    x_user = np.asarray(inputs["x_user"])
    x_item = np.asarray(inputs["x_item"])
    hu0 = np.asarray(inputs["emb_user"], F32)[x_user]
    hi0 = np.asarray(inputs["emb_item"], F32)[x_item]
    W1l = np.asarray(inputs["W1l"], F32)
    W1r = np.asarray(inputs["W1r"], F32)
    b1 = np.asarray(inputs["b1"], F32)
    W2l = np.asarray(inputs["W2l"], F32)
    W2r = np.asarray(inputs["W2r"], F32)
    b2 = np.asarray(inputs["b2"], F32)
    predW = np.asarray(inputs["pred_W"], F32)
    predb = np.asarray(inputs["pred_b"], F32)
    ug_src = np.asarray(inputs["ug_src"], np.int64)
    ug_dst = np.asarray(inputs["ug_dst"], np.int64)
    ui_src = np.asarray(inputs["ui_src"], np.int64)
    ui_dst = np.asarray(inputs["ui_dst"], np.int64)
    gi_src = np.asarray(inputs["gi_src"], np.int64)
    gi_dst = np.asarray(inputs["gi_dst"], np.int64)

    w_ug_g = (1.0 / np.maximum(np.bincount(ug_dst, minlength=NG), 1)).astype(F32)
    w_gi_g = (1.0 / np.maximum(np.bincount(gi_src, minlength=NG), 1)).astype(F32)
    w_ui_i = (1.0 / np.maximum(np.bincount(ui_dst, minlength=NI), 1)).astype(F32)
    w_ui_u = (1.0 / np.maximum(np.bincount(ui_src, minlength=NU), 1)).astype(F32)

    # user table [USH_P, 256]: cols 0:128 = hu0 shard; 128:256 = hu1 (device)
    ugt = np.zeros((W, USH_P, 2 * H), BF16)
    # item shard table [ISH_P, 256]: cols 0:128 = hi0 shard; 128:256 = hi1
    git = np.zeros((W, ISH_P, 2 * H), BF16)
    # full item table (layer1 features only) for i2u gathers
    ite = np.zeros((NI_P, H), BF16)
    for c in range(W):
        ugt[c, :USH, :H] = hu0[c * USH:(c + 1) * USH].astype(BF16)
        git[c, :ISH, :H] = hi0[c * ISH:(c + 1) * ISH].astype(BF16)
        ite[c * ISH_P:c * ISH_P + ISH] = hi0[c * ISH:(c + 1) * ISH].astype(BF16)

    d_ug = Dir("ug", N_GT, False, SEG_UG)
    per = []
    for c in range(W):
        m = (ug_src >= c * USH) & (ug_src < (c + 1) * USH)
        per.append(((ug_src[m] - c * USH).astype(np.int16),
                    ug_dst[m], w_ug_g[ug_dst[m]]))
    d_ug.build(per)

    # gi is dense enough (25K edges onto 2560x5120 per core) that a
    # host-built adjacency block beats per-edge gathers 4x.
    agi = np.zeros((W, ISH_P, NG_P), BF16)
    for c in range(W):
        m = (gi_dst >= c * ISH) & (gi_dst < (c + 1) * ISH)
        il = (gi_dst[m] - c * ISH).astype(np.int64)
        g = gi_src[m]
        acc = np.zeros((ISH_P, NG_P), F32)
        np.add.at(acc, (il, g), w_gi_g[g])
        agi[c] = acc.astype(BF16)

    d_uii = Dir("uii", N_IT, True, SEG_SM)   # u2i: dst = items (padded)
    d_iu = Dir("iu", N_UT, True, SEG_SM)     # i2u: dst = local users
    per_uii, per_iu = [], []
    for c in range(W):
        m = (ui_src >= c * USH) & (ui_src < (c + 1) * USH)
        us, ud = ui_src[m], ui_dst[m]
        per_uii.append(((us - c * USH).astype(np.int16),
                        _pad_item(ud), w_ui_i[ud]))
        per_iu.append((_pad_item(ud).astype(np.int16),
                       (us - c * USH), w_ui_u[us]))
    d_uii.build(per_uii)
    d_iu.build(per_iu)

    wts = np.stack([
        W1l[0], W1l[5],                 # og1: u2g, i2g
        W1l[2], W1r[2] + W1r[4],        # oi1: u2i agg, dense
        W1l[3], W1r[1] + W1r[3],        # ou1: i2u agg, dense
        W2l[0], W2l[5], W2r[0] + W2r[5]  # og2
    ]).astype(BF16)
    biases = np.stack([b1[0] + b1[5], b1[1] + b1[3],
                       b2[0] + b2[5], np.zeros(H, F32)], axis=1).astype(F32)
    btile_i1 = np.broadcast_to((b1[2] + b1[4]).astype(BF16), (128, H)).copy()
    ident = np.eye(128, dtype=BF16)
    iota = np.broadcast_to(np.arange(128, dtype=np.float32),
                           (128, 128)).astype(BF16)

    predW_sh = np.zeros((W, H, ISH_P), BF16)
    predb_sh = np.zeros((W, N_IST, 128), F32)
    for c in range(W):
        predW_sh[c][:, :ISH] = predW[:, c * ISH:(c + 1) * ISH].astype(BF16)
        pb = np.zeros(ISH_P, F32)
        pb[:ISH] = predb[c * ISH:(c + 1) * ISH]
        predb_sh[c] = pb.reshape(N_IST, 128)

    in_maps = []
    for c in range(W):
        in_maps.append({
            "ugt": ugt[c], "git": git[c], "ite": ite,
            "wts": wts, "biases": biases, "btile_i1": btile_i1,
            "ident": ident, "iota": iota,
            "predw": predW_sh[c], "predb": predb_sh[c],
            "ug_idx": d_ug.idx[c], "ug_dstl": d_ug.dstl[c], "ug_wv": d_ug.wv[c],
            "agi": agi[c],
            "uii_idx": d_uii.idx[c], "uii_dstl": d_uii.dstl[c],
            "uii_wv": d_uii.wv[c],
            "iu_idx": d_iu.idx[c], "iu_dstl": d_iu.dstl[c], "iu_wv": d_iu.wv[c],
        })
    return in_maps, {"ug": d_ug, "uii": d_uii, "iu": d_iu}


def _build(struct):
    d_ug, d_uii, d_iu = struct["ug"], struct["uii"], struct["iu"]
    nc = bacc.Bacc("TRN2", target_bir_lowering=False, num_swdge_queues=4)
    bf = mybir.dt.bfloat16
    f32 = mybir.dt.float32
    i16 = mybir.dt.int16

    P = {}

    def param(name, shape, dt):
        P[name] = nc.declare_dram_parameter(name, list(shape), dt,
                                            isOutput=False)
        return P[name]

    ugt = param("ugt", [USH_P, 2 * H], bf)
    git = param("git", [ISH_P, 2 * H], bf)
    ite = param("ite", [NI_P, H], bf)
    wts = param("wts", [9, 128, 128], bf)
    biases = param("biases", [128, 4], f32)
    btile_i1 = param("btile_i1", [128, H], bf)
    ident_d = param("ident", [128, 128], bf)
    iota_d = param("iota", [128, 128], bf)
    predw = param("predw", [H, ISH_P], bf)
    predb = param("predb", [N_IST, 128], f32)
    agi_d = param("agi", [ISH_P, NG_P], bf)
    for nm, d in (("ug", d_ug), ("uii", d_uii), ("iu", d_iu)):
        C = d.total_chunks
        param(f"{nm}_idx", [128, C * 8], i16)
        param(f"{nm}_dstl", [128, C], bf)
        param(f"{nm}_wv", [128, C], bf)
    outp = nc.declare_dram_parameter("out", [ISH_P, NG], bf, isOutput=True)

    with tile.TileContext(nc) as tc:
        with (
            tc.tile_pool(name="cst", bufs=1) as cst,
            tc.tile_pool(name="gp", bufs=2) as gp,
            tc.tile_pool(name="sp", bufs=3) as sp,
            tc.tile_pool(name="st", bufs=2) as stp,
            tc.tile_pool(name="big", bufs=2) as bigp,
            tc.tile_pool(name="psum", bufs=1, space="PSUM") as psum,
            tc.tile_pool(name="dram", bufs=1, space="DRAM") as dram,
        ):
            wt_sb = []
            for k in range(9):
                t = cst.tile([128, 128], bf, tag=f"w{k}")
                nc.sync.dma_start(t[:], wts[k])
                wt_sb.append(t)
            (W_og_u, W_og_i, W_oi_a, W_oi_d, W_ou_a, W_ou_d,
             W_og2_u, W_og2_i, W_og2_d) = wt_sb
            bias_sb = cst.tile([128, 4], f32, tag="bias")
            nc.sync.dma_start(bias_sb[:], biases[:])
            bti_sb = cst.tile([128, H], bf, tag="bti")
            nc.sync.dma_start(bti_sb[:], btile_i1[:])
            ident_sb = cst.tile([128, 128], bf, tag="ident")
            nc.sync.dma_start(ident_sb[:], ident_d[:])
            predb_sb = cst.tile([128, N_IST], f32, tag="predb")
            nc.sync.dma_start(predb_sb[:], predb[:].rearrange("a b -> b a"))
            iota_sb = cst.tile([128, 128], bf, tag="iota")
            nc.sync.dma_start(iota_sb[:], iota_d[:])

            darr = {}
            for nm, d in (("ug", d_ug), ("uii", d_uii), ("iu", d_iu)):
                C = d.total_chunks
                ti_ = cst.tile([128, C * 8], i16, tag=f"{nm}_idx")
                nc.sync.dma_start(ti_[:], P[f"{nm}_idx"][:])
                td = cst.tile([128, C], bf, tag=f"{nm}_dstl")
                nc.sync.dma_start(td[:], P[f"{nm}_dstl"][:])
                tw = cst.tile([128, C], bf, tag=f"{nm}_wv")
                nc.sync.dma_start(tw[:], P[f"{nm}_wv"][:])
                darr[nm] = (ti_, td, tw)

            # ogT chunk layout: 5 chunks of 1024 groups; chunk c occupies
            # cols [c*2048, (c+1)*2048): first 1024 = og1, second = og2.
            # Lets each chunk AllReduce independently as soon as its
            # contributions are complete.
            NCH = NG_P // 1024
            ogT = bigp.tile([128, 2 * NG_P], bf, tag="big", name="ogT",
                            bufs=1)
            nc.vector.memset(ogT[:], 0.0)

            def og1_sl(col, w=128):
                c = col // 1024
                return ogT[:, c * 2048 + col % 1024:
                           c * 2048 + col % 1024 + w]

            def og2_sl(col, w=128):
                c = col // 1024
                return ogT[:, c * 2048 + 1024 + col % 1024:
                           c * 2048 + 1024 + col % 1024 + w]

            hg1T = cst.tile([128, NG_P], bf, tag="hg1T")
            repT = cst.tile([128, NG_P], bf, tag="repT")

            hiT_full = cst.tile([128, ISH_P], bf, tag="hiTf")
            nc.sync.dma_start(hiT_full[:], git[:, 0:H], transpose=True)
            predw_sb = cst.tile([128, ISH_P], bf, tag="predw_sb")
            nc.sync.dma_start(predw_sb[:], predw[:])

            aroi_in = dram.tile([NI_P, H], bf)
            rs_oi = dram.tile([ISH_P, H], bf)
            arc_in = [dram.tile([128, 2048], bf, tag=f"arcin{c}",
                                name=f"arcin{c}") for c in range(NCH)]
            arc_out = [dram.tile([128, 2048], bf, tag=f"arcout{c}",
                                 name=f"arcout{c}") for c in range(NCH)]

            qctr = [0]

            def segsum(d, table_ap, elem_size, elem_step, width, out_cb,
                       between=None):
                idx_sb, dstl_sb, wv_sb = darr[d.name]
                for si, (cs, cn, seg_tiles) in enumerate(d.segments):
                    if between is not None:
                        between(si)
                    gt = gp.tile([128, d.seg_chunks, width], bf,
                                 tag=f"gath{width}",
                                 bufs=(4 if width == H else 3))
                    n_idx = cn * 128
                    nc.gpsimd.dma_gather(
                        gt[:, :cn, :], table_ap,
                        idx_sb[:, cs * 8:(cs + cn) * 8],
                        n_idx, n_idx, elem_size, elem_step=elem_step,
                        single_packet=False, queue_num=qctr[0] % 4)
                    qctr[0] += 1
                    oh = gp.tile([128, d.seg_chunks, 128], bf, tag="oh",
                                 bufs=2)
                    iota_b = (iota_sb[:].rearrange("p (o k) -> p o k", o=1)
                              .to_broadcast((128, cn, 128)))
                    dstl_b = (dstl_sb[:, cs:cs + cn]
                              .rearrange("p (c o) -> p c o", o=1)
                              .to_broadcast((128, cn, 128)))
                    wv_b = (wv_sb[:, cs:cs + cn]
                            .rearrange("p (c o) -> p c o", o=1)
                            .to_broadcast((128, cn, 128)))
                    nc.vector.tensor_tensor(oh[:, :cn, :], iota_b, dstl_b,
                                            AluOpType.is_equal)
                    nc.vector.tensor_tensor(oh[:, :cn, :], oh[:, :cn, :],
                                            wv_b, AluOpType.mult)
                    for (ti, ofs_t, nct) in seg_tiles:
                        out_cb(ti, gt, oh, ofs_t - cs, nct)

            # ---------- Phase 1: i2u -> hu1 (local users) ----------
            scope_p1 = tc.spectator_scope("p1_i2u")
            scope_p1.__enter__()
            hu_stage = [None]

            huTg_cache = [None]

            def get_huT(ti):
                g8 = ti // 8
                if huTg_cache[0] is None or huTg_cache[0][0] != g8:
                    n_t = min(8, N_UT - g8 * 8)
                    tl = sp.tile([128, 1024], bf, tag="huTg", name="huTg", bufs=2)
                    nc.sync.dma_start(
                        tl[:, :n_t * 128],
                        ugt[g8 * 1024:g8 * 1024 + n_t * 128, 0:H],
                        transpose=True)
                    huTg_cache[0] = (g8, tl)
                return huTg_cache[0][1][:, (ti % 8) * 128:(ti % 8 + 1) * 128]

            def cb_ou(ti, gt, oh, lc0, nct):
                ps = psum.tile([128, 128], f32, tag="agg", bufs=2)
                for j in range(nct):
                    nc.tensor.matmul(ps[:], gt[:, lc0 + j, :], oh[:, lc0 + j, :],
                                     start=(j == 0), stop=(j == nct - 1))
                aggT = sp.tile([128, 128], bf, tag="aggT", bufs=4)
                nc.scalar.activation(aggT[:], ps[:],
                                     mybir.ActivationFunctionType.Copy)
                pw = psum.tile([128, 128], f32, tag="w", bufs=2)
                nc.tensor.matmul(pw[:], W_ou_a[:], aggT[:], start=True,
                                 stop=False)
                nc.tensor.matmul(pw[:], W_ou_d[:], get_huT(ti), start=False,
                                 stop=True)
                ouT = sp.tile([128, 128], bf, tag="ouT", bufs=4)
                nc.scalar.activation(ouT[:], pw[:],
                                     mybir.ActivationFunctionType.Relu,
                                     bias=bias_sb[:, 1:2])
                ptr = psum.tile([128, 128], bf, tag="w", bufs=2)
                nc.tensor.transpose(ptr[:], ouT[:], ident_sb[:])
                g, s = ti // 16, ti % 16
                if hu_stage[0] is None:
                    hu_stage[0] = stp.tile([128, 16, 128], bf, tag="hust", name="hust")
                nc.vector.tensor_copy(hu_stage[0][:, s, :], ptr[:])
                if s == 15 or ti == N_UT - 1:
                    n_g = s + 1
                    nc.sync.dma_start(
                        ugt[g * 2048:g * 2048 + n_g * 128, H:2 * H]
                        .rearrange("(k p) h -> p k h", p=128),
                        hu_stage[0][:, :n_g, :])
                    hu_stage[0] = None

            segsum(d_iu, ite[:], H, H, H, cb_ou)
            scope_p1.__exit__(None, None, None)

            # ---------- Phase 2: u2i -> oi1 partial (item-major) ----------
            scope_p2 = tc.spectator_scope("p2_u2i")
            scope_p2.__enter__()
            oi_stage = [None]

            def cb_oi(ti, gt, oh, lc0, nct):
                ps = psum.tile([128, 128], f32, tag="agg", bufs=2)
                for j in range(nct):
                    nc.tensor.matmul(ps[:], gt[:, lc0 + j, :], oh[:, lc0 + j, :],
                                     start=(j == 0), stop=(j == nct - 1))
                aggT = sp.tile([128, 128], bf, tag="aggT", bufs=4)
                nc.scalar.activation(aggT[:], ps[:],
                                     mybir.ActivationFunctionType.Copy)
                pw = psum.tile([128, 128], f32, tag="w", bufs=2)
                nc.tensor.matmul(pw[:], aggT[:], W_oi_a[:], start=True,
                                 stop=True)
                g, s = ti // 16, ti % 16
                if oi_stage[0] is None:
                    oi_stage[0] = stp.tile([128, 16, 128], bf, tag="oist", name="oist")
                nc.vector.tensor_copy(oi_stage[0][:, s, :], pw[:])
                if s == 15 or ti == N_IT - 1:
                    n_g = s + 1
                    nc.sync.dma_start(
                        aroi_in[g * 2048:g * 2048 + n_g * 128, :]
                        .rearrange("(k p) h -> p k h", p=128),
                        oi_stage[0][:, :n_g, :])
                    oi_stage[0] = None

            segsum(d_uii, ugt[:, 0:H], H, 2 * H, H, cb_oi)
            scope_p2.__exit__(None, None, None)

            with tc.spectator_scope("p2b_rs"):
                nc.gpsimd.collective_compute(
                    "ReduceScatter", AluOpType.add,
                    replica_groups=[list(range(W))],
                    ins=[aroi_in.opt()], outs=[rs_oi.opt()])

            # ---------- Phases 4a: u2g both layers (needs hu1 only) ------
            def make_cb_g(W_l1, W_l2):
                def cb(ti, gt, oh, lc0, nct):
                    ps0 = psum.tile([128, 128], f32, tag="agg", bufs=2)
                    ps1 = psum.tile([128, 128], f32, tag="agg1", bufs=2)
                    for j in range(nct):
                        nc.tensor.matmul(ps0[:], gt[:, lc0 + j, 0:H],
                                         oh[:, lc0 + j, :],
                                         start=(j == 0), stop=(j == nct - 1))
                        nc.tensor.matmul(ps1[:], gt[:, lc0 + j, H:2 * H],
                                         oh[:, lc0 + j, :],
                                         start=(j == 0), stop=(j == nct - 1))
                    a0 = sp.tile([128, 128], bf, tag="aggT", bufs=4)
                    nc.scalar.activation(a0[:], ps0[:],
                                         mybir.ActivationFunctionType.Copy)
                    a1 = sp.tile([128, 128], bf, tag="aggT2", bufs=4)
                    nc.scalar.activation(a1[:], ps1[:],
                                         mybir.ActivationFunctionType.Copy)
                    pw = psum.tile([128, 128], f32, tag="w", bufs=2)
                    nc.tensor.matmul(pw[:], W_l1[:], a0[:], start=True,
                                     stop=True)
                    s1 = og1_sl(ti * 128)
                    nc.vector.tensor_tensor(s1, s1, pw[:], AluOpType.add)
                    pw2 = psum.tile([128, 128], f32, tag="w", bufs=2)
                    nc.tensor.matmul(pw2[:], W_l2[:], a1[:], start=True,
                                     stop=True)
                    s2 = og2_sl(ti * 128)
                    nc.vector.tensor_tensor(s2, s2, pw2[:], AluOpType.add)
                return cb

            # ---------- Phase 3: hi1 = relu(rs + dense + b) ----------
            # emitted lazily inside the u2g interleave (after RS has had a
            # few u2g segments' time to complete) to avoid stalling PE.
            def p3_hi1():
                for t in range(N_IST):
                    rs_sb = sp.tile([128, 128], bf, tag="rs")
                    nc.sync.dma_start(rs_sb[:],
                                      rs_oi[t * 128:(t + 1) * 128, :])
                    pd = psum.tile([128, 128], f32, tag="w", bufs=2)
                    nc.tensor.matmul(pd[:],
                                     hiT_full[:, t * 128:(t + 1) * 128],
                                     W_oi_d[:], start=True, stop=True)
                    t1 = sp.tile([128, 128], bf, tag="t1")
                    nc.vector.tensor_tensor(t1[:], rs_sb[:], pd[:],
                                            AluOpType.add)
                    t2 = sp.tile([128, 128], bf, tag="t2")
                    nc.vector.tensor_tensor(t2[:], t1[:], bti_sb[:],
                                            AluOpType.add)
                    hi1_t = sp.tile([128, 128], bf, tag="hi1")
                    nc.scalar.activation(hi1_t[:], t2[:],
                                         mybir.ActivationFunctionType.Relu)
                    nc.sync.dma_start(git[t * 128:(t + 1) * 128, H:2 * H],
                                      hi1_t[:])

            # ---------- Phase 5: i2g both layers via dense adjacency ----
            # mean_T[H, groups] = sum_t git_tile[K=item,H].T @ A[item, groups];
            # then (mean @ W) folds in afterwards per 1024-col block.
            # Emitted per-jg, interleaved into the u2g segment loop so the
            # PE work fills the gather-drain window; loads go on the
            # Activation engine's HWDGE queue to keep Sync free for oh.
            def p5_block(jg):
                p0 = psum.tile([128, 1024], f32, tag="agg", bufs=2)
                p1 = psum.tile([128, 1024], f32, tag="agg", bufs=2)
                for t in range(N_IST):
                    gsb = sp.tile([128, 2 * H], bf, tag="gisb", bufs=2)
                    nc.scalar.dma_start(gsb[:], git[t * 128:(t + 1) * 128, :])
                    asb = sp.tile([128, 1024], bf, tag="agisb", bufs=2)
                    nc.scalar.dma_start(
                        asb[:],
                        agi_d[t * 128:(t + 1) * 128,
                              jg * 1024:(jg + 1) * 1024])
                    for q in range(2):
                        nc.tensor.matmul(
                            p0[:, q * 512:(q + 1) * 512], gsb[:, 0:H],
                            asb[:, q * 512:(q + 1) * 512],
                            start=(t == 0), stop=(t == N_IST - 1))
                        nc.tensor.matmul(
                            p1[:, q * 512:(q + 1) * 512], gsb[:, H:2 * H],
                            asb[:, q * 512:(q + 1) * 512],
                            start=(t == 0), stop=(t == N_IST - 1))
                # aggregate-T is now in psum; fold W via aggT copy + W-MM
                for k in range(8):
                    a0 = sp.tile([128, 128], bf, tag="aggT", bufs=4)
                    nc.scalar.activation(a0[:], p0[:, k * 128:(k + 1) * 128],
                                         mybir.ActivationFunctionType.Copy)
                    a1 = sp.tile([128, 128], bf, tag="aggT2", bufs=4)
                    nc.scalar.activation(a1[:], p1[:, k * 128:(k + 1) * 128],
                                         mybir.ActivationFunctionType.Copy)
                    pw = psum.tile([128, 128], f32, tag="w", bufs=2)
                    nc.tensor.matmul(pw[:], W_og_i[:], a0[:], start=True,
                                     stop=True)
                    s1 = og1_sl(jg * 1024 + k * 128)
                    nc.vector.tensor_tensor(s1, s1, pw[:], AluOpType.add)
                    pw2 = psum.tile([128, 128], f32, tag="w", bufs=2)
                    nc.tensor.matmul(pw2[:], W_og2_i[:], a1[:], start=True,
                                     stop=True)
                    s2 = og2_sl(jg * 1024 + k * 128)
                    nc.vector.tensor_tensor(s2, s2, pw2[:], AluOpType.add)

            # ---------- Phase 4a: u2g gathers (sequential) ----------
            scope_p4 = tc.spectator_scope("p4_u2g")
            scope_p4.__enter__()
            segsum(d_ug, ugt[:], 2 * H, 2 * H, 2 * H,
                   make_cb_g(W_og_u, W_og2_u))
            scope_p4.__exit__(None, None, None)

            # ---------- Phase 3 then 5, with per-chunk AllReduce ----------
            scope_p35 = tc.spectator_scope("p35")
            scope_p35.__enter__()
            p3_hi1()
            for c in range(NCH):
                p5_block(c)
                nc.sync.dma_start(arc_in[c][:],
                                  ogT[:, c * 2048:(c + 1) * 2048])
                nc.gpsimd.collective_compute(
                    "AllReduce", AluOpType.add,
                    replica_groups=[list(range(W))],
                    ins=[arc_in[c].opt()], outs=[arc_out[c].opt()])
            scope_p35.__exit__(None, None, None)

            # ----- Phases 6+7: per-chunk fold + predictor pipeline -----
            # For each 1024-group chunk: load its AR result, relu og1 into
            # hg1T, fold the og2 dense term, relu into repT, then run the
            # predictor for those columns across all item tiles. Chunk c's
            # compute overlaps chunk c+1's AllReduce transfer.
            scope_p67 = tc.spectator_scope("p67_fold_pred")
            scope_p67.__enter__()
            for c in range(NCH):
                arc_sb = stp.tile([128, 2048], bf, tag="arcsb", bufs=2)
                nc.sync.dma_start(arc_sb[:], arc_out[c][:])
                g0 = c * 1024
                nc.scalar.activation(hg1T[:, g0:g0 + 1024], arc_sb[:, 0:1024],
                                     mybir.ActivationFunctionType.Relu,
                                     bias=bias_sb[:, 0:1])
                pfold = psum.tile([128, 1024], f32, tag="agg", bufs=2)
                for q in range(2):
                    nc.tensor.matmul(pfold[:, q * 512:(q + 1) * 512],
                                     W_og2_d[:],
                                     hg1T[:, g0 + q * 512:g0 + (q + 1) * 512],
                                     start=True, stop=True)
                tt = sp.tile([128, 1024], bf, tag="o2t")
                nc.vector.tensor_tensor(tt[:], arc_sb[:, 1024:2048],
                                        pfold[:], AluOpType.add)
                nc.scalar.activation(repT[:, g0:g0 + 1024], tt[:],
                                     mybir.ActivationFunctionType.Relu,
                                     bias=bias_sb[:, 2:3])
                wj = min(1024, NG - g0)
                for t in range(N_IST):
                    pf = psum.tile([128, 1024], f32, tag="agg", bufs=2)
                    for q in range((wj + 511) // 512):
                        wq = min(512, wj - q * 512)
                        nc.tensor.matmul(
                            pf[:, q * 512:q * 512 + wq],
                            predw_sb[:, t * 128:(t + 1) * 128],
                            repT[:, g0 + q * 512:g0 + q * 512 + wq],
                            start=True, stop=True)
                    stg = stp.tile([128, 1024], bf, tag="fstage", bufs=3)
                    nc.vector.tensor_scalar(
                        stg[:, :wj], pf[:, :wj],
                        predb_sb[:, t:t + 1], None, AluOpType.add)
                    nc.sync.dma_start(
                        outp[t * 128:(t + 1) * 128, g0:g0 + wj],
                        stg[:, :wj])
            scope_p67.__exit__(None, None, None)
    nc.compile()
    return nc


def kernel(**inputs):
    in_maps, struct = _prep(inputs)
    nc = _build(struct)
    res = run_bass_kernel_spmd(nc, in_maps, list(range(W)))
    parts = [res.results[c]["out"][:ISH] for c in range(W)]
    full = np.concatenate(parts, axis=0).astype(np.float32)  # [NI, NG]
    return full.T  # [NG, NI] zero-copy view

